# revision 9
# baseline (speedup 1.0000x reference)
"""Trainium2 Bass kernel for 16-head causal RoPE attention (B=1, L=4096, D=1024).

Distribution: tensor-parallel over heads - each of the 8 cores owns 2 heads
(128 q/k/v dims) and computes a partial output projection; the host sums the
8 partial [1024, 4096] bf16 outputs and transposes back to [1, 4096, 1024].

v2 dataflow (all-bf16 operands, f32 PSUM accumulation):
  xT [1024, L] bf16 streamed -> q/k paired psum via bf16 matmuls (FWL fast
  weight loads); RoPE via u = qk*sin, PE permutation matmul P@u, then
  qk*cos + (P@u) on DVE, operating on [128, 2, 512] q||k paired tiles.
  v transposed per 128-chunk on PE into vno [kv, kvc, 2, 65] bf16 lhsT with
  fused ones column (softmax denominator rides the av matmul as row 64).
  scoresT [128 kv, 512 q] per head via row-tiled matmul PAIRS: both heads
  computed concurrently in PE row-groups 0-1 / 2-3 (tile_position (0,0) /
  (64,0)), K=64 each.
  exp on ACT ([128, 2, 512] per head per 2-kv-chunk group); causal masks
  applied post-exp on DVE for the 4 diagonal chunks (paired [128, 2, 512]).
  Normalization: denominators from av psum row 64 -> reciprocal_approx_fast
  on [2, 512] -> partition-broadcast via a K=2 PE matmul -> DVE muls.
  Final projection woT.T @ outT -> yT [1024, L] bf16 partial, streamed out.
"""

import numpy as np

N_HEAD = 16
HEAD_DIM = 64
HIDDEN = 1024
N_CORES = 8
ROPE_BASE = 10000.0

_CACHE = {}


def _build(L):
    import concourse.bass as bass
    import concourse.tile as tile
    import concourse.mybir as mybir
    from concourse import bacc
    from concourse.masks import make_identity

    F32 = mybir.dt.float32
    F32R = mybir.dt.float32r
    BF16 = mybir.dt.bfloat16
    Exp = mybir.ActivationFunctionType.Exp

    LC = L // 512          # number of 512-wide q chunks
    KVC = L // 128         # number of 128-wide kv chunks
    HC = HIDDEN // 128     # hidden contraction chunks

    nc = bacc.Bacc("TRN2", target_bir_lowering=False, debug=False,
                   num_devices=N_CORES)

    xT_d = nc.dram_tensor("xT", [HIDDEN, L], BF16, kind="ExternalInput")
    wqT_d = nc.dram_tensor("wqT", [HIDDEN, 128], BF16, kind="ExternalInput")
    wkT_d = nc.dram_tensor("wkT", [HIDDEN, 128], BF16, kind="ExternalInput")
    wvT_d = nc.dram_tensor("wvT", [HIDDEN, 128], BF16, kind="ExternalInput")
    woT_d = nc.dram_tensor("woT", [128, HIDDEN], BF16, kind="ExternalInput")
    cosT_d = nc.dram_tensor("cosT", [128, L], BF16, kind="ExternalInput")
    sinT_d = nc.dram_tensor("sinT", [128, L], BF16, kind="ExternalInput")
    masks_d = nc.dram_tensor("masks", [128, 4 * 512], BF16, kind="ExternalInput")
    pmat_d = nc.dram_tensor("pmat", [128, 128], BF16, kind="ExternalInput")
    yT_d = nc.dram_tensor("yT", [HIDDEN, L], BF16, kind="ExternalOutput")

    with tile.TileContext(nc) as tc:
        with tc.tile_pool(name="big", bufs=1) as big, \
             tc.tile_pool(name="vno_p", bufs=1) as vno_p, \
             tc.tile_pool(name="w_p", bufs=1) as w_p, \
             tc.tile_pool(name="sm2", bufs=2) as sm2, \
             tc.tile_pool(name="sm3", bufs=2) as sm3, \
             tc.tile_pool(name="xt_p", bufs=2) as xt_p, \
             tc.tile_pool(name="att_p", bufs=4) as att_p, \
             tc.tile_pool(name="ps_st", bufs=2, space="PSUM") as ps_st, \
             tc.tile_pool(name="ps_ph", bufs=1, space="PSUM") as ps_ph, \
             tc.tile_pool(name="ps_av", bufs=2, space="PSUM") as ps_av:

            # ---- constants / weights ----
            wq_sb = w_p.tile([128, HC, 128], BF16, tag="wq")
            wk_sb = w_p.tile([128, HC, 128], BF16, tag="wk")
            wv_sb = w_p.tile([128, HC, 128], BF16, tag="wv")
            for w_sb, w_d in ((wq_sb, wqT_d), (wk_sb, wkT_d), (wv_sb, wvT_d)):
                nc.sync.dma_start(
                    out=w_sb,
                    in_=w_d.ap().rearrange("(c p) m -> p c m", p=128))
            wo_sb = w_p.tile([128, HIDDEN], BF16, tag="wo")
            masks_sb = w_p.tile([128, 4, 512], BF16, tag="masks")
            pmat_sb = w_p.tile([128, 128], BF16, tag="pmat")
            nc.gpsimd.dma_start(out=pmat_sb, in_=pmat_d.ap())
            nc.gpsimd.dma_start(out=wo_sb, in_=woT_d.ap())
            nc.gpsimd.dma_start(
                out=masks_sb, in_=masks_d.ap().rearrange("p (j n) -> p j n", j=4))
            ident = w_p.tile([128, 128], BF16, tag="ident")
            make_identity(nc, ident)

            # broadcast lhsT for the K=1 denominator-replication matmuls
            onesbc = w_p.tile([1, 64], BF16, tag="onesbc")
            nc.vector.memset(onesbc, 1.0)

            # v in lhsT layout with fused ones columns:
            # [kv_in_chunk, kvc, head, 65]; col 64 of each head slot = 1.0
            vno = vno_p.tile([128, KVC, 2, 65], BF16, tag="vno")
            nc.vector.memset(vno[:, :, :, 64:65], 1.0)

            # q||k rope'd, feature-on-partition: [:, 0, :] = q, [:, 1, :] = k
            qkro = big.tile([128, 2, L], BF16, tag="qkro")

            def phase1(n):
                ns = slice(n * 512, (n + 1) * 512)
                xt = xt_p.tile([128, HC, 512], BF16, tag="xt")
                nc.sync.dma_start(
                    out=xt,
                    in_=xT_d.ap()[:, ns].rearrange("(c p) m -> p c m", p=128))
                cs2 = sm2.tile([128, 2, 512], BF16, tag="cs")
                sn2 = sm2.tile([128, 2, 512], BF16, tag="sn")
                for t_sb, t_d in ((cs2, cosT_d), (sn2, sinT_d)):
                    for j in range(2):
                        nc.sync.dma_start(out=t_sb[:, j, :], in_=t_d.ap()[:, ns])

                # q and k projections into one paired psum tile
                ps_qk = ps_ph.tile([128, 2, 512], F32, tag="ph")
                for j, w_sb in ((0, wq_sb), (1, wk_sb)):
                    for k in range(HC):
                        nc.tensor.matmul(ps_qk[:, j, :], w_sb[:, k, :],
                                         xt[:, k, :],
                                         start=(k == 0), stop=(k == HC - 1))
                qk = sm3.tile([128, 2, 512], BF16, tag="qk")
                nc.vector.tensor_copy(qk, ps_qk)

                # rope: qkro = qk*cos + P @ (qk*sin)
                u = sm3.tile([128, 2, 512], BF16, tag="u")
                nc.vector.tensor_mul(u, qk, sn2)
                ps_sw = ps_ph.tile([128, 2, 512], F32, tag="ph")
                for j in range(2):
                    nc.tensor.matmul(ps_sw[:, j, :], pmat_sb, u[:, j, :])
                t1 = sm3.tile([128, 2, 512], BF16, tag="t1")
                nc.vector.tensor_mul(t1, qk, cs2)
                nc.vector.tensor_add(qkro[:, :, ns], t1, ps_sw)

                # v projection + transpose into vno
                ps_vt = ps_ph.tile([128, 2, 512], F32, tag="ph")
                for k in range(HC):
                    nc.tensor.matmul(ps_vt[:, 0, :], wv_sb[:, k, :], xt[:, k, :],
                                     start=(k == 0), stop=(k == HC - 1))
                vt = sm3.tile([128, 512], BF16, tag="vt")
                nc.vector.tensor_copy(vt, ps_vt[:, 0, :])
                ps_tr = ps_vt[:, 1, 0:256].bitcast(BF16)   # [128, 512] bf16 view
                for j in range(4):
                    nc.tensor.transpose(ps_tr[:, j * 128:(j + 1) * 128],
                                        vt[:, j * 128:(j + 1) * 128], ident)
                nc.vector.tensor_copy(
                    vno[:, 4 * n:4 * n + 4, :, 0:64],
                    ps_tr.rearrange("p (c h d) -> p c h d", c=4, h=2))

            def attention(qc):
                qs = slice(qc * 512, (qc + 1) * 512)
                n_kc = 4 * (qc + 1)          # kv chunks (128 each), causal
                n_g = n_kc // 2              # groups of 2 kv chunks
                av0 = ps_av.tile([65, 512], F32, tag="av")
                av1 = ps_av.tile([65, 512], F32, tag="av")
                avs = [av0, av1]

                def st_exp(g):
                    # row-tiled score pairs: both heads concurrently
                    st0 = ps_st.tile([128, 2, 512], F32, tag="st")
                    st1 = ps_st.tile([128, 2, 512], F32, tag="st")
                    for j in range(2):
                        kc = 2 * g + j
                        ks = slice(kc * 128, (kc + 1) * 128)
                        nc.tensor.matmul(
                            st0[:, j, :], qkro[0:64, 1, ks], qkro[0:64, 0, qs],
                            tile_position=(0, 0))
                        nc.tensor.matmul(
                            st1[:, j, :], qkro[64:128, 1, ks], qkro[64:128, 0, qs],
                            tile_position=(64, 0))
                    atts = []
                    for h, st in ((0, st0), (1, st1)):
                        att = att_p.tile([128, 2, 512], BF16, tag="att")
                        nc.scalar.activation(att, st, Exp)
                        if g >= n_g - 2:  # diagonal band: apply causal masks
                            mj = 2 * g - 4 * qc
                            nc.vector.tensor_mul(att, att,
                                                 masks_sb[:, mj:mj + 2, :])
                        atts.append(att)
                    return atts

                def av_mms(g_prev, atts_prev):
                    for h in range(2):
                        for j in range(2):
                            kc = 2 * g_prev + j
                            nc.tensor.matmul(
                                avs[h], vno[:, kc, h, :],
                                atts_prev[h][:, j, :],
                                start=(kc == 0), stop=(kc == n_kc - 1))

                pending = None
                for g in range(n_g):
                    atts = st_exp(g)
                    if pending is not None:
                        av_mms(*pending)
                    pending = (g, atts)
                av_mms(*pending)

                # normalization: rden = 1/den, broadcast via PE, multiply
                den = sm2.tile([1, 2, 512], F32, tag="den")
                for h in range(2):
                    nc.vector.tensor_copy(den[0:1, h, :], avs[h][64:65, :])
                rden = sm2.tile([1, 2, 512], F32, tag="rden")
                nc.vector.reciprocal_approx_fast(rden, den)
                rdenb = sm2.tile([1, 2, 512], BF16, tag="rdenb")
                nc.vector.tensor_copy(rdenb, rden)
                bci = ps_st.tile([128, 2, 512], F32, tag="st")
                nc.tensor.matmul(bci[0:64, 0, :], onesbc, rdenb[0:1, 0, :],
                                 tile_position=(0, 0))
                nc.tensor.matmul(bci[64:128, 0, :], onesbc, rdenb[0:1, 1, :],
                                 tile_position=(0, 64))
                bci_sb = sm2.tile([128, 512], BF16, tag="bci")
                nc.vector.tensor_copy(bci_sb, bci[:, 0, :])
                outT = sm2.tile([128, 512], BF16, tag="outT")
                nc.vector.tensor_mul(outT[0:64, :], avs[0][0:64, :],
                                     bci_sb[0:64, :])
                nc.vector.tensor_mul(outT[64:128, :], avs[1][0:64, :],
                                     bci_sb[64:128, :])

                # final projection for this q chunk, 2 output chunks at a time
                for e in range(HC // 2):
                    ps_y = ps_st.tile([128, 2, 512], F32, tag="st")
                    for j in range(2):
                        nc.tensor.matmul(
                            ps_y[:, j, :],
                            wo_sb[:, (2 * e + j) * 128:(2 * e + j + 1) * 128],
                            outT)
                    y_sb = sm2.tile([128, 2, 512], BF16, tag="y")
                    nc.vector.tensor_copy(y_sb, ps_y)
                    nc.sync.dma_start(
                        out=yT_d.ap()[2 * e * 128:(2 * e + 2) * 128, qs]
                            .rearrange("(c p) m -> p c m", p=128),
                        in_=y_sb)

            phase1(0)
            phase1(1)
            for n in range(2, LC):
                attention(n - 2)
                phase1(n)
            attention(LC - 2)
            attention(LC - 1)

    nc.compile()
    return nc


def _host_prep(x, wq, wk, wv, wo, L):
    """Build per-core input maps (numpy only)."""
    import ml_dtypes
    bf16 = ml_dtypes.bfloat16

    x2 = np.ascontiguousarray(x.reshape(L, HIDDEN))
    xT = np.ascontiguousarray(x2.T).astype(bf16)

    # rope tables, transposed + duplicated for the two heads on each core
    inv_freq = 1.0 / (ROPE_BASE ** (np.arange(0, HEAD_DIM, 2, dtype=np.float64)
                                    / HEAD_DIM))
    freqs = np.arange(L, dtype=np.float64)[:, None] * inv_freq[None, :]
    emb = np.concatenate([freqs, freqs], axis=-1)          # [L, 64]
    cosT = np.cos(emb).T.astype(np.float32)                # [64, L]
    sinT = np.sin(emb).T.astype(np.float32)
    cosT2 = np.ascontiguousarray(np.concatenate([cosT, cosT], axis=0)).astype(bf16)
    sinT2 = np.ascontiguousarray(np.concatenate([sinT, sinT], axis=0)).astype(bf16)

    # causal masks for the 4 diagonal kv chunks of each 512-q chunk
    kv = np.arange(128)[:, None]
    q = np.arange(512)[None, :]
    masks = np.concatenate(
        [(q >= j * 128 + kv).astype(bf16) for j in range(4)], axis=1)
    masks = np.ascontiguousarray(masks)                    # [128, 2048] bf16

    # rotate-half permutation (as matmul lhsT), block-diag for 2 heads
    P = np.zeros((64, 64), np.float32)
    P[np.arange(32) + 32, np.arange(32)] = -1.0
    P[np.arange(32), np.arange(32) + 32] = 1.0
    pmat = np.zeros((128, 128), np.float32)
    pmat[0:64, 0:64] = P
    pmat[64:128, 64:128] = P
    pmat = pmat.astype(bf16)

    in_maps = []
    for c in range(N_CORES):
        rows = slice(c * 128, (c + 1) * 128)
        in_maps.append({
            "xT": xT,
            "wqT": np.ascontiguousarray(
                wq[rows, :].T * np.float32(1.0 / 8.0)).astype(bf16),
            "wkT": np.ascontiguousarray(wk[rows, :].T).astype(bf16),
            "wvT": np.ascontiguousarray(wv[rows, :].T).astype(bf16),
            "woT": np.ascontiguousarray(wo[:, rows].T).astype(bf16),
            "cosT": cosT2,
            "sinT": sinT2,
            "masks": masks,
            "pmat": pmat,
        })
    return in_maps


def _ensure_profile_hook():
    """The agent image's antenv lacks axon_hooks; recreate it from the boot
    package so trace=True can capture NTFF profiles."""
    import sys, types
    try:
        from antenv.axon_hooks import get_axon_ntff_profile_hook  # noqa: F401
        return
    except ImportError:
        pass
    try:
        from trn_agent_boot.trn_boot import _ntff_profile_via_ctypes
        hook = _ntff_profile_via_ctypes('/opt/axon/libaxon_pjrt.so')
    except Exception:
        hook = None
    mod = types.ModuleType("antenv.axon_hooks")
    mod.get_axon_ntff_profile_hook = lambda: hook
    mod.set_axon_ntff_profile_hook = lambda h: None
    sys.modules["antenv.axon_hooks"] = mod


def _run(x, wq, wk, wv, wo, trace=False, trace_cores=None):
    from concourse.bass_utils import run_bass_kernel_spmd

    if trace:
        _ensure_profile_hook()

    B, L, D = x.shape
    assert (B, D) == (1, HIDDEN)
    if L not in _CACHE:
        _CACHE[L] = _build(L)
    nc = _CACHE[L]
    in_maps = _host_prep(np.asarray(x, np.float32), wq, wk, wv, wo, L)
    res = run_bass_kernel_spmd(
        nc, in_maps, core_ids=list(range(N_CORES)),
        trace=trace, trace_cores=trace_cores)
    acc = np.zeros((HIDDEN, L), np.float64)
    for r in res.results:
        acc += r["yT"].astype(np.float64)
    y = np.ascontiguousarray(acc.T.astype(np.float32)).reshape(1, L, HIDDEN)
    return y, res


def kernel(x, wq, wk, wv, wo):
    y, _ = _run(np.asarray(x), np.asarray(wq), np.asarray(wk),
                np.asarray(wv), np.asarray(wo))
    return y


# revision 10
# speedup vs baseline: 1.1248x; 1.1248x over previous
"""Trainium2 Bass kernel for 16-head causal RoPE attention (B=1, L=4096, D=1024).

Distribution: tensor-parallel over heads - each of the 8 cores owns 2 heads
(128 q/k/v dims) and computes a partial output projection; the host sums the
8 partial [1024, 4096] bf16 outputs and transposes back to [1, 4096, 1024].

v3 dataflow (all-bf16 operands, f32 PSUM accumulation), two serial phases
with scoped PSUM pools:

Phase 1 (projections + RoPE), 8-bank PSUM ring:
  xT [1024, L] bf16 -> q/k paired psum via bf16 matmuls (FWL weight loads);
  RoPE as qk*cos + P@(qk*sin) on [128, 2, 512] q||k paired tiles; v
  transposed per 128-chunk on PE into vno [kv, kvc, 2, 65] bf16 lhsT with a
  fused ones column (softmax denominator rides the av matmul as row 64).

Phase 2 (attention), per-kv-chunk pipeline, 3 deep:
  one [128, 2(head), 512] psum tile per kv chunk; both heads' scoresT
  computed CONCURRENTLY as a row-tiled matmul pair (tile_position (0,0) /
  (64,0), K=64 each); ONE exp ACTIVATE per kv chunk covering both heads
  (frees both banks atomically so the next pair becomes ready as a unit);
  ONE causal-mask multiply per diagonal chunk (masks duplicated per head).
  Normalization: denominators from av psum row 64 -> reciprocal_approx_fast
  on [1, 2, 512] -> partition-broadcast via two col-tiled K=1 PE matmuls ->
  DVE muls. Final projection woT.T @ outT -> yT [1024, L] bf16 partial.
"""

import numpy as np

N_HEAD = 16
HEAD_DIM = 64
HIDDEN = 1024
N_CORES = 8
ROPE_BASE = 10000.0

_CACHE = {}


def _build(L):
    import concourse.bass as bass
    import concourse.tile as tile
    import concourse.mybir as mybir
    from concourse import bacc
    from concourse.masks import make_identity

    F32 = mybir.dt.float32
    F32R = mybir.dt.float32r
    BF16 = mybir.dt.bfloat16
    Exp = mybir.ActivationFunctionType.Exp

    LC = L // 512          # number of 512-wide q chunks
    KVC = L // 128         # number of 128-wide kv chunks
    HC = HIDDEN // 128     # hidden contraction chunks

    nc = bacc.Bacc("TRN2", target_bir_lowering=False, debug=False,
                   num_devices=N_CORES)

    xT_d = nc.dram_tensor("xT", [HIDDEN, L], BF16, kind="ExternalInput")
    wqT_d = nc.dram_tensor("wqT", [HIDDEN, 128], BF16, kind="ExternalInput")
    wkT_d = nc.dram_tensor("wkT", [HIDDEN, 128], BF16, kind="ExternalInput")
    wvT_d = nc.dram_tensor("wvT", [HIDDEN, 128], BF16, kind="ExternalInput")
    woT_d = nc.dram_tensor("woT", [128, HIDDEN], BF16, kind="ExternalInput")
    cosT_d = nc.dram_tensor("cosT", [128, L], BF16, kind="ExternalInput")
    sinT_d = nc.dram_tensor("sinT", [128, L], BF16, kind="ExternalInput")
    masks_d = nc.dram_tensor("masks", [128, 4 * 2 * 512], BF16,
                             kind="ExternalInput")
    pmat_d = nc.dram_tensor("pmat", [128, 128], BF16, kind="ExternalInput")
    yT_d = nc.dram_tensor("yT", [HIDDEN, L], BF16, kind="ExternalOutput")

    with tile.TileContext(nc) as tc:
        with tc.tile_pool(name="big", bufs=1) as big, \
             tc.tile_pool(name="vno_p", bufs=1) as vno_p, \
             tc.tile_pool(name="w_p", bufs=1) as w_p, \
             tc.tile_pool(name="sm2", bufs=2) as sm2, \
             tc.tile_pool(name="sm3", bufs=2) as sm3, \
             tc.tile_pool(name="xt_p", bufs=2) as xt_p, \
             tc.tile_pool(name="att_p", bufs=4) as att_p:

            # ---- constants / weights ----
            wq_sb = w_p.tile([128, HC, 128], BF16, tag="wq")
            wk_sb = w_p.tile([128, HC, 128], BF16, tag="wk")
            wv_sb = w_p.tile([128, HC, 128], BF16, tag="wv")
            for w_sb, w_d in ((wq_sb, wqT_d), (wk_sb, wkT_d), (wv_sb, wvT_d)):
                nc.sync.dma_start(
                    out=w_sb,
                    in_=w_d.ap().rearrange("(c p) m -> p c m", p=128))
            wo_sb = w_p.tile([128, HIDDEN], BF16, tag="wo")
            masks_sb = w_p.tile([128, 4, 2, 512], BF16, tag="masks")
            pmat_sb = w_p.tile([128, 128], BF16, tag="pmat")
            nc.gpsimd.dma_start(out=pmat_sb, in_=pmat_d.ap())
            nc.gpsimd.dma_start(out=wo_sb, in_=woT_d.ap())
            nc.gpsimd.dma_start(
                out=masks_sb,
                in_=masks_d.ap().rearrange("p (j h n) -> p j h n", j=4, h=2))
            ident = w_p.tile([128, 128], BF16, tag="ident")
            make_identity(nc, ident)

            # broadcast lhsT for the K=1 denominator-replication matmuls
            onesbc = w_p.tile([1, 64], BF16, tag="onesbc")
            nc.vector.memset(onesbc, 1.0)

            # v in lhsT layout with fused ones columns:
            # [kv_in_chunk, kvc, head, 65]; col 64 of each head slot = 1.0
            vno = vno_p.tile([128, KVC, 2, 65], BF16, tag="vno")
            nc.vector.memset(vno[:, :, :, 64:65], 1.0)

            # q||k rope'd, feature-on-partition: [:, 0, :] = q, [:, 1, :] = k
            qkro = big.tile([128, 2, L], BF16, tag="qkro")

            def phase1(n, ps_ph):
                ns = slice(n * 512, (n + 1) * 512)
                xt = xt_p.tile([128, HC, 512], BF16, tag="xt")
                nc.sync.dma_start(
                    out=xt,
                    in_=xT_d.ap()[:, ns].rearrange("(c p) m -> p c m", p=128))
                cs2 = sm2.tile([128, 2, 512], BF16, tag="cs")
                sn2 = sm2.tile([128, 2, 512], BF16, tag="sn")
                for t_sb, t_d in ((cs2, cosT_d), (sn2, sinT_d)):
                    for j in range(2):
                        nc.sync.dma_start(out=t_sb[:, j, :], in_=t_d.ap()[:, ns])

                # q and k projections into one paired psum tile
                ps_qk = ps_ph.tile([128, 2, 512], F32, tag="ph")
                for j, w_sb in ((0, wq_sb), (1, wk_sb)):
                    for k in range(HC):
                        nc.tensor.matmul(ps_qk[:, j, :], w_sb[:, k, :],
                                         xt[:, k, :],
                                         start=(k == 0), stop=(k == HC - 1))
                qk = sm3.tile([128, 2, 512], BF16, tag="qk")
                nc.vector.tensor_copy(qk, ps_qk)

                # rope: qkro = qk*cos + P @ (qk*sin)
                u = sm3.tile([128, 2, 512], BF16, tag="u")
                nc.vector.tensor_mul(u, qk, sn2)
                ps_sw = ps_ph.tile([128, 2, 512], F32, tag="ph")
                for j in range(2):
                    nc.tensor.matmul(ps_sw[:, j, :], pmat_sb, u[:, j, :])
                t1 = sm3.tile([128, 2, 512], BF16, tag="t1")
                nc.vector.tensor_mul(t1, qk, cs2)
                nc.vector.tensor_add(qkro[:, :, ns], t1, ps_sw)

                # v projection + transpose into vno
                ps_vt = ps_ph.tile([128, 2, 512], F32, tag="ph")
                for k in range(HC):
                    nc.tensor.matmul(ps_vt[:, 0, :], wv_sb[:, k, :], xt[:, k, :],
                                     start=(k == 0), stop=(k == HC - 1))
                vt = sm3.tile([128, 512], BF16, tag="vt")
                nc.vector.tensor_copy(vt, ps_vt[:, 0, :])
                ps_tr = ps_vt[:, 1, 0:256].bitcast(BF16)   # [128, 512] bf16 view
                for j in range(4):
                    nc.tensor.transpose(ps_tr[:, j * 128:(j + 1) * 128],
                                        vt[:, j * 128:(j + 1) * 128], ident)
                nc.vector.tensor_copy(
                    vno[:, 4 * n:4 * n + 4, :, 0:64],
                    ps_tr.rearrange("p (c h d) -> p c h d", c=4, h=2))

            def attention(qc, ps_st, ps_av):
                qs = slice(qc * 512, (qc + 1) * 512)
                n_kc = 4 * (qc + 1)          # kv chunks (128 each), causal
                av0 = ps_av.tile([65, 512], F32, tag="av")
                av1 = ps_av.tile([65, 512], F32, tag="av")
                avs = [av0, av1]

                def st_exp(kc):
                    # row-tiled score pair: both heads concurrently, one tile
                    st = ps_st.tile([128, 2, 512], F32, tag="st")
                    ks = slice(kc * 128, (kc + 1) * 128)
                    nc.tensor.matmul(
                        st[:, 0, :], qkro[0:64, 1, ks], qkro[0:64, 0, qs],
                        tile_position=(0, 0))
                    nc.tensor.matmul(
                        st[:, 1, :], qkro[64:128, 1, ks], qkro[64:128, 0, qs],
                        tile_position=(64, 0))
                    att = att_p.tile([128, 2, 512], BF16, tag="att")
                    nc.scalar.activation(att, st, Exp)
                    if kc >= 4 * qc:  # diagonal band: apply causal mask
                        nc.vector.tensor_mul(att, att,
                                             masks_sb[:, kc - 4 * qc, :, :])
                    return att

                def av_mm(kc, att):
                    for h in range(2):
                        nc.tensor.matmul(
                            avs[h], vno[:, kc, h, :], att[:, h, :],
                            start=(kc == 0), stop=(kc == n_kc - 1))

                pending = None
                for kc in range(n_kc):
                    if pending is not None:
                        av_mm(*pending)
                    pending = (kc, st_exp(kc))
                av_mm(*pending)

                # normalization: rden = 1/den, broadcast via PE, multiply
                den = sm2.tile([1, 2, 512], F32, tag="den")
                for h in range(2):
                    nc.vector.tensor_copy(den[0:1, h, :], avs[h][64:65, :])
                rden = sm2.tile([1, 2, 512], F32, tag="rden")
                nc.vector.reciprocal_approx_fast(rden, den)
                rdenb = sm2.tile([1, 2, 512], BF16, tag="rdenb")
                nc.vector.tensor_copy(rdenb, rden)
                bci = ps_st.tile([128, 2, 512], F32, tag="st")
                nc.tensor.matmul(bci[0:64, 0, :], onesbc, rdenb[0:1, 0, :],
                                 tile_position=(0, 0))
                nc.tensor.matmul(bci[64:128, 0, :], onesbc, rdenb[0:1, 1, :],
                                 tile_position=(0, 64))
                bci_sb = sm2.tile([128, 512], BF16, tag="bci")
                nc.vector.tensor_copy(bci_sb, bci[:, 0, :])
                outT = sm2.tile([128, 512], BF16, tag="outT")
                nc.vector.tensor_mul(outT[0:64, :], avs[0][0:64, :],
                                     bci_sb[0:64, :])
                nc.vector.tensor_mul(outT[64:128, :], avs[1][0:64, :],
                                     bci_sb[64:128, :])

                # final projection for this q chunk, 2 output chunks at a time
                for e in range(HC // 2):
                    ps_y = ps_st.tile([128, 2, 512], F32, tag="st")
                    for j in range(2):
                        nc.tensor.matmul(
                            ps_y[:, j, :],
                            wo_sb[:, (2 * e + j) * 128:(2 * e + j + 1) * 128],
                            outT)
                    y_sb = sm2.tile([128, 2, 512], BF16, tag="y")
                    nc.vector.tensor_copy(y_sb, ps_y)
                    nc.sync.dma_start(
                        out=yT_d.ap()[2 * e * 128:(2 * e + 2) * 128, qs]
                            .rearrange("(c p) m -> p c m", p=128),
                        in_=y_sb)

            with tc.tile_pool(name="ps_ph", bufs=4, space="PSUM") as ps_ph:
                for n in range(LC):
                    phase1(n, ps_ph)
            with tc.tile_pool(name="ps_st", bufs=3, space="PSUM") as ps_st, \
                 tc.tile_pool(name="ps_av", bufs=2, space="PSUM") as ps_av:
                for qc in range(LC):
                    attention(qc, ps_st, ps_av)

    nc.compile()
    return nc


def _host_prep(x, wq, wk, wv, wo, L):
    """Build per-core input maps (numpy only)."""
    import ml_dtypes
    bf16 = ml_dtypes.bfloat16

    x2 = np.ascontiguousarray(x.reshape(L, HIDDEN))
    xT = np.ascontiguousarray(x2.T).astype(bf16)

    # rope tables, transposed + duplicated for the two heads on each core
    inv_freq = 1.0 / (ROPE_BASE ** (np.arange(0, HEAD_DIM, 2, dtype=np.float64)
                                    / HEAD_DIM))
    freqs = np.arange(L, dtype=np.float64)[:, None] * inv_freq[None, :]
    emb = np.concatenate([freqs, freqs], axis=-1)          # [L, 64]
    cosT = np.cos(emb).T.astype(np.float32)                # [64, L]
    sinT = np.sin(emb).T.astype(np.float32)
    cosT2 = np.ascontiguousarray(np.concatenate([cosT, cosT], axis=0)).astype(bf16)
    sinT2 = np.ascontiguousarray(np.concatenate([sinT, sinT], axis=0)).astype(bf16)

    # causal masks for the 4 diagonal kv chunks, duplicated per head:
    # [128, j=4, h=2, 512]
    kv = np.arange(128)[:, None]
    q = np.arange(512)[None, :]
    m = np.stack([(q >= j * 128 + kv).astype(bf16) for j in range(4)], axis=1)
    masks = np.ascontiguousarray(
        np.repeat(m[:, :, None, :], 2, axis=2).reshape(128, 4 * 2 * 512))

    # rotate-half permutation (as matmul lhsT), block-diag for 2 heads
    P = np.zeros((64, 64), np.float32)
    P[np.arange(32) + 32, np.arange(32)] = -1.0
    P[np.arange(32), np.arange(32) + 32] = 1.0
    pmat = np.zeros((128, 128), np.float32)
    pmat[0:64, 0:64] = P
    pmat[64:128, 64:128] = P
    pmat = pmat.astype(bf16)

    in_maps = []
    for c in range(N_CORES):
        rows = slice(c * 128, (c + 1) * 128)
        in_maps.append({
            "xT": xT,
            "wqT": np.ascontiguousarray(
                wq[rows, :].T * np.float32(1.0 / 8.0)).astype(bf16),
            "wkT": np.ascontiguousarray(wk[rows, :].T).astype(bf16),
            "wvT": np.ascontiguousarray(wv[rows, :].T).astype(bf16),
            "woT": np.ascontiguousarray(wo[:, rows].T).astype(bf16),
            "cosT": cosT2,
            "sinT": sinT2,
            "masks": masks,
            "pmat": pmat,
        })
    return in_maps


def _ensure_profile_hook():
    """The agent image's antenv lacks axon_hooks; recreate it from the boot
    package so trace=True can capture NTFF profiles."""
    import sys, types
    try:
        from antenv.axon_hooks import get_axon_ntff_profile_hook  # noqa: F401
        return
    except ImportError:
        pass
    try:
        from trn_agent_boot.trn_boot import _ntff_profile_via_ctypes
        hook = _ntff_profile_via_ctypes('/opt/axon/libaxon_pjrt.so')
    except Exception:
        hook = None
    mod = types.ModuleType("antenv.axon_hooks")
    mod.get_axon_ntff_profile_hook = lambda: hook
    mod.set_axon_ntff_profile_hook = lambda h: None
    sys.modules["antenv.axon_hooks"] = mod


def _run(x, wq, wk, wv, wo, trace=False, trace_cores=None):
    from concourse.bass_utils import run_bass_kernel_spmd

    if trace:
        _ensure_profile_hook()

    B, L, D = x.shape
    assert (B, D) == (1, HIDDEN)
    if L not in _CACHE:
        _CACHE[L] = _build(L)
    nc = _CACHE[L]
    in_maps = _host_prep(np.asarray(x, np.float32), wq, wk, wv, wo, L)
    res = run_bass_kernel_spmd(
        nc, in_maps, core_ids=list(range(N_CORES)),
        trace=trace, trace_cores=trace_cores)
    acc = np.zeros((HIDDEN, L), np.float64)
    for r in res.results:
        acc += r["yT"].astype(np.float64)
    y = np.ascontiguousarray(acc.T.astype(np.float32)).reshape(1, L, HIDDEN)
    return y, res


def kernel(x, wq, wk, wv, wo):
    y, _ = _run(np.asarray(x), np.asarray(wq), np.asarray(wk),
                np.asarray(wv), np.asarray(wo))
    return y


# revision 15
# speedup vs baseline: 1.2918x; 1.1485x over previous
"""Trainium2 Bass kernel for 16-head causal RoPE attention (B=1, L=4096, D=1024).

Distribution: tensor-parallel over heads - each of the 8 cores owns 2 heads
(128 q/k/v dims) and computes a partial output projection; the host sums the
8 partial [1024, 4096] bf16 outputs and transposes back to [1, 4096, 1024].

v3 dataflow (all-bf16 operands, f32 PSUM accumulation), two serial phases
with scoped PSUM pools:

Phase 1 (projections + RoPE), 8-bank PSUM ring:
  xT [1024, L] bf16 -> q/k paired psum via bf16 matmuls (FWL weight loads);
  RoPE as qk*cos + P@(qk*sin) on [128, 2, 512] q||k paired tiles; v
  transposed per 128-chunk on PE into vno [kv, kvc, 2, 65] bf16 lhsT with a
  fused ones column (softmax denominator rides the av matmul as row 64).

Phase 2 (attention), per-kv-chunk pipeline, 3 deep:
  one [128, 2(head), 512] psum tile per kv chunk; both heads' scoresT
  computed CONCURRENTLY as a row-tiled matmul pair (tile_position (0,0) /
  (64,0), K=64 each); ONE exp ACTIVATE per kv chunk covering both heads
  (frees both banks atomically so the next pair becomes ready as a unit);
  ONE causal-mask multiply per diagonal chunk (masks duplicated per head).
  Normalization: denominators from av psum row 64 -> reciprocal_approx_fast
  on [1, 2, 512] -> partition-broadcast via two col-tiled K=1 PE matmuls ->
  DVE muls. Final projection woT.T @ outT -> yT [1024, L] bf16 partial.
"""

import numpy as np

N_HEAD = 16
HEAD_DIM = 64
HIDDEN = 1024
N_CORES = 8
ROPE_BASE = 10000.0

_CACHE = {}


def _build(L):
    import concourse.bass as bass
    import concourse.tile as tile
    import concourse.mybir as mybir
    from concourse import bacc
    from concourse.masks import make_identity

    F32 = mybir.dt.float32
    F32R = mybir.dt.float32r
    BF16 = mybir.dt.bfloat16
    Exp = mybir.ActivationFunctionType.Exp

    LC = L // 512          # number of 512-wide q chunks
    KVC = L // 128         # number of 128-wide kv chunks
    HC = HIDDEN // 128     # hidden contraction chunks

    nc = bacc.Bacc("TRN2", target_bir_lowering=False, debug=False,
                   num_devices=N_CORES)

    xT_d = nc.dram_tensor("xT", [HIDDEN, L], BF16, kind="ExternalInput")
    wqT_d = nc.dram_tensor("wqT", [HIDDEN, 128], BF16, kind="ExternalInput")
    wkT_d = nc.dram_tensor("wkT", [HIDDEN, 128], BF16, kind="ExternalInput")
    wvT_d = nc.dram_tensor("wvT", [HIDDEN, 128], BF16, kind="ExternalInput")
    woT_d = nc.dram_tensor("woT", [128, HIDDEN], BF16, kind="ExternalInput")
    cosT_d = nc.dram_tensor("cosT", [128, L], BF16, kind="ExternalInput")
    sinT_d = nc.dram_tensor("sinT", [128, L], BF16, kind="ExternalInput")
    masks_d = nc.dram_tensor("masks", [128, 4 * 2 * 512], BF16,
                             kind="ExternalInput")
    pmat_d = nc.dram_tensor("pmat", [128, 128], BF16, kind="ExternalInput")
    yT_d = nc.dram_tensor("yT", [HIDDEN, L], BF16, kind="ExternalOutput")

    with tile.TileContext(nc) as tc:
        with tc.tile_pool(name="big", bufs=1) as big, \
             tc.tile_pool(name="vno_p", bufs=1) as vno_p, \
             tc.tile_pool(name="w_p", bufs=1) as w_p, \
             tc.tile_pool(name="sm2", bufs=2) as sm2, \
             tc.tile_pool(name="sm3", bufs=2) as sm3, \
             tc.tile_pool(name="xt_p", bufs=2) as xt_p, \
             tc.tile_pool(name="att_p", bufs=4) as att_p:

            # ---- constants / weights ----
            wq_sb = w_p.tile([128, HC, 128], BF16, tag="wq")
            wk_sb = w_p.tile([128, HC, 128], BF16, tag="wk")
            wv_sb = w_p.tile([128, HC, 128], BF16, tag="wv")
            for w_sb, w_d in ((wq_sb, wqT_d), (wk_sb, wkT_d), (wv_sb, wvT_d)):
                nc.sync.dma_start(
                    out=w_sb,
                    in_=w_d.ap().rearrange("(c p) m -> p c m", p=128))
            wo_sb = w_p.tile([128, HIDDEN], BF16, tag="wo")
            masks_sb = w_p.tile([128, 4, 2, 512], BF16, tag="masks")
            pmat_sb = w_p.tile([128, 128], BF16, tag="pmat")
            nc.gpsimd.dma_start(out=pmat_sb, in_=pmat_d.ap())
            nc.gpsimd.dma_start(out=wo_sb, in_=woT_d.ap())
            nc.gpsimd.dma_start(
                out=masks_sb,
                in_=masks_d.ap().rearrange("p (j h n) -> p j h n", j=4, h=2))
            ident = w_p.tile([128, 128], BF16, tag="ident")
            make_identity(nc, ident)

            # broadcast lhsT for the K=1 denominator-replication matmuls
            onesbc = w_p.tile([1, 64], BF16, tag="onesbc")
            nc.vector.memset(onesbc, 1.0)

            # v in lhsT layout with fused ones columns:
            # [kv_in_chunk, kvc, head, 65]; col 64 of each head slot = 1.0
            vno = vno_p.tile([128, KVC, 2, 65], BF16, tag="vno")
            nc.vector.memset(vno[:, :, :, 64:65], 1.0)

            # q||k rope'd, feature-on-partition: [:, 0, :] = q, [:, 1, :] = k
            qkro = big.tile([128, 2, L], BF16, tag="qkro")

            def phase1(n, ps_ph):
                ns = slice(n * 512, (n + 1) * 512)
                xt = xt_p.tile([128, HC, 512], BF16, tag="xt")
                nc.sync.dma_start(
                    out=xt,
                    in_=xT_d.ap()[:, ns].rearrange("(c p) m -> p c m", p=128))
                cs2 = sm2.tile([128, 2, 512], BF16, tag="cs")
                sn2 = sm2.tile([128, 2, 512], BF16, tag="sn")
                for t_sb, t_d in ((cs2, cosT_d), (sn2, sinT_d)):
                    for j in range(2):
                        nc.gpsimd.dma_start(out=t_sb[:, j, :],
                                            in_=t_d.ap()[:, ns])

                # q and k projections into one paired psum tile
                ps_qk = ps_ph.tile([128, 2, 512], F32, tag="ph")
                for j, w_sb in ((0, wq_sb), (1, wk_sb)):
                    for k in range(HC):
                        nc.tensor.matmul(ps_qk[:, j, :], w_sb[:, k, :],
                                         xt[:, k, :],
                                         start=(k == 0), stop=(k == HC - 1))
                # v projection (keeps PE dense while rope's DVE ops catch up)
                ps_vt = ps_ph.tile([128, 2, 512], F32, tag="ph")
                for k in range(HC):
                    nc.tensor.matmul(ps_vt[:, 0, :], wv_sb[:, k, :], xt[:, k, :],
                                     start=(k == 0), stop=(k == HC - 1))

                qk = sm3.tile([128, 2, 512], BF16, tag="qk")
                nc.vector.tensor_copy(qk, ps_qk)

                # rope: qkro = qk*cos + P @ (qk*sin)
                u = sm3.tile([128, 2, 512], BF16, tag="u")
                nc.vector.tensor_mul(u, qk, sn2)
                ps_sw = ps_ph.tile([128, 2, 512], F32, tag="ph")
                for j in range(2):
                    nc.tensor.matmul(ps_sw[:, j, :], pmat_sb, u[:, j, :])
                t1 = sm3.tile([128, 2, 512], BF16, tag="t1")
                nc.vector.tensor_mul(t1, qk, cs2)
                nc.vector.tensor_add(qkro[:, :, ns], t1, ps_sw)

                # v transpose into vno
                vt = sm3.tile([128, 512], BF16, tag="vt")
                nc.vector.tensor_copy(vt, ps_vt[:, 0, :])
                ps_tr = ps_vt[:, 1, 0:256].bitcast(BF16)   # [128, 512] bf16 view
                for j in range(4):
                    nc.tensor.transpose(ps_tr[:, j * 128:(j + 1) * 128],
                                        vt[:, j * 128:(j + 1) * 128], ident)
                nc.vector.tensor_copy(
                    vno[:, 4 * n:4 * n + 4, :, 0:64],
                    ps_tr.rearrange("p (c h d) -> p c h d", c=4, h=2))

            def attention(qc, ps_st, ps_av, ps_pr):
                qs = slice(qc * 512, (qc + 1) * 512)
                n_kc = 4 * (qc + 1)          # kv chunks (128 each), causal
                av0 = ps_av.tile([65, 512], F32, tag="av")
                av1 = ps_av.tile([65, 512], F32, tag="av")
                avs = [av0, av1]

                def st_exp(kc):
                    # row-tiled score pair: both heads concurrently, one tile
                    st = ps_st.tile([128, 2, 512], F32, tag="st")
                    ks = slice(kc * 128, (kc + 1) * 128)
                    nc.tensor.matmul(
                        st[:, 0, :], qkro[0:64, 1, ks], qkro[0:64, 0, qs],
                        tile_position=(0, 0))
                    nc.tensor.matmul(
                        st[:, 1, :], qkro[64:128, 1, ks], qkro[64:128, 0, qs],
                        tile_position=(64, 0))
                    att = att_p.tile([128, 2, 512], BF16, tag="att")
                    nc.scalar.activation(att, st, Exp)
                    if kc >= 4 * qc:  # diagonal band: apply causal mask
                        nc.vector.tensor_mul(att, att,
                                             masks_sb[:, kc - 4 * qc, :, :])
                    return att

                def av_mm(kc, att):
                    for h in range(2):
                        nc.tensor.matmul(
                            avs[h], vno[:, kc, h, :], att[:, h, :],
                            start=(kc == 0), stop=(kc == n_kc - 1))

                pending = None
                for kc in range(n_kc):
                    if pending is not None:
                        av_mm(*pending)
                    pending = (kc, st_exp(kc))
                av_mm(*pending)

                # normalization: rden = 1/den, broadcast via PE, multiply
                den = sm2.tile([1, 2, 512], F32, tag="den")
                for h in range(2):
                    nc.vector.tensor_copy(den[0:1, h, :], avs[h][64:65, :])
                rden = sm2.tile([1, 2, 512], F32, tag="rden")
                nc.vector.reciprocal_approx_fast(rden, den)
                rdenb = sm2.tile([1, 2, 512], BF16, tag="rdenb")
                nc.vector.tensor_copy(rdenb, rden)
                bci = ps_pr.tile([128, 2, 512], F32, tag="pr")
                nc.tensor.matmul(bci[0:64, 0, :], onesbc, rdenb[0:1, 0, :],
                                 tile_position=(0, 0))
                nc.tensor.matmul(bci[64:128, 0, :], onesbc, rdenb[0:1, 1, :],
                                 tile_position=(0, 64))
                bci_sb = sm2.tile([128, 512], BF16, tag="bci")
                nc.vector.tensor_copy(bci_sb, bci[:, 0, :])
                outT = sm2.tile([128, 512], BF16, tag="outT")
                nc.vector.tensor_mul(outT[0:64, :], avs[0][0:64, :],
                                     bci_sb[0:64, :])
                nc.vector.tensor_mul(outT[64:128, :], avs[1][0:64, :],
                                     bci_sb[64:128, :])

                # final projection for this q chunk, 2 output chunks at a time
                for e in range(HC // 2):
                    ps_y = ps_pr.tile([128, 2, 512], F32, tag="pr")
                    for j in range(2):
                        nc.tensor.matmul(
                            ps_y[:, j, :],
                            wo_sb[:, (2 * e + j) * 128:(2 * e + j + 1) * 128],
                            outT)
                    y_sb = sm2.tile([128, 2, 512], BF16, tag="y")
                    nc.vector.tensor_copy(y_sb, ps_y)
                    nc.sync.dma_start(
                        out=yT_d.ap()[2 * e * 128:(2 * e + 2) * 128, qs]
                            .rearrange("(c p) m -> p c m", p=128),
                        in_=y_sb)

            with tc.tile_pool(name="ps_ph", bufs=4, space="PSUM") as ps_ph:
                for n in range(LC):
                    phase1(n, ps_ph)
            with tc.tile_pool(name="ps_st", bufs=2, space="PSUM") as ps_st, \
                 tc.tile_pool(name="ps_av", bufs=2, space="PSUM") as ps_av, \
                 tc.tile_pool(name="ps_pr", bufs=1, space="PSUM") as ps_pr:
                for qc in range(LC):
                    attention(qc, ps_st, ps_av, ps_pr)

    nc.compile()
    return nc


def _host_prep(x, wq, wk, wv, wo, L):
    """Build per-core input maps (numpy only)."""
    import ml_dtypes
    bf16 = ml_dtypes.bfloat16

    x2 = np.ascontiguousarray(x.reshape(L, HIDDEN))
    xT = np.ascontiguousarray(x2.T).astype(bf16)

    # rope tables, transposed + duplicated for the two heads on each core
    inv_freq = 1.0 / (ROPE_BASE ** (np.arange(0, HEAD_DIM, 2, dtype=np.float64)
                                    / HEAD_DIM))
    freqs = np.arange(L, dtype=np.float64)[:, None] * inv_freq[None, :]
    emb = np.concatenate([freqs, freqs], axis=-1)          # [L, 64]
    cosT = np.cos(emb).T.astype(np.float32)                # [64, L]
    sinT = np.sin(emb).T.astype(np.float32)
    cosT2 = np.ascontiguousarray(np.concatenate([cosT, cosT], axis=0)).astype(bf16)
    sinT2 = np.ascontiguousarray(np.concatenate([sinT, sinT], axis=0)).astype(bf16)

    # causal masks for the 4 diagonal kv chunks, duplicated per head:
    # [128, j=4, h=2, 512]
    kv = np.arange(128)[:, None]
    q = np.arange(512)[None, :]
    m = np.stack([(q >= j * 128 + kv).astype(bf16) for j in range(4)], axis=1)
    masks = np.ascontiguousarray(
        np.repeat(m[:, :, None, :], 2, axis=2).reshape(128, 4 * 2 * 512))

    # rotate-half permutation (as matmul lhsT), block-diag for 2 heads
    P = np.zeros((64, 64), np.float32)
    P[np.arange(32) + 32, np.arange(32)] = -1.0
    P[np.arange(32), np.arange(32) + 32] = 1.0
    pmat = np.zeros((128, 128), np.float32)
    pmat[0:64, 0:64] = P
    pmat[64:128, 64:128] = P
    pmat = pmat.astype(bf16)

    in_maps = []
    for c in range(N_CORES):
        rows = slice(c * 128, (c + 1) * 128)
        in_maps.append({
            "xT": xT,
            "wqT": np.ascontiguousarray(
                wq[rows, :].T * np.float32(1.0 / 8.0)).astype(bf16),
            "wkT": np.ascontiguousarray(wk[rows, :].T).astype(bf16),
            "wvT": np.ascontiguousarray(wv[rows, :].T).astype(bf16),
            "woT": np.ascontiguousarray(wo[:, rows].T).astype(bf16),
            "cosT": cosT2,
            "sinT": sinT2,
            "masks": masks,
            "pmat": pmat,
        })
    return in_maps


def _ensure_profile_hook():
    """The agent image's antenv lacks axon_hooks; recreate it from the boot
    package so trace=True can capture NTFF profiles."""
    import sys, types
    try:
        from antenv.axon_hooks import get_axon_ntff_profile_hook  # noqa: F401
        return
    except ImportError:
        pass
    try:
        from trn_agent_boot.trn_boot import _ntff_profile_via_ctypes
        hook = _ntff_profile_via_ctypes('/opt/axon/libaxon_pjrt.so')
    except Exception:
        hook = None
    mod = types.ModuleType("antenv.axon_hooks")
    mod.get_axon_ntff_profile_hook = lambda: hook
    mod.set_axon_ntff_profile_hook = lambda h: None
    sys.modules["antenv.axon_hooks"] = mod


def _run(x, wq, wk, wv, wo, trace=False, trace_cores=None):
    from concourse.bass_utils import run_bass_kernel_spmd

    if trace:
        _ensure_profile_hook()

    B, L, D = x.shape
    assert (B, D) == (1, HIDDEN)
    if L not in _CACHE:
        _CACHE[L] = _build(L)
    nc = _CACHE[L]
    in_maps = _host_prep(np.asarray(x, np.float32), wq, wk, wv, wo, L)
    res = run_bass_kernel_spmd(
        nc, in_maps, core_ids=list(range(N_CORES)),
        trace=trace, trace_cores=trace_cores)
    acc = np.zeros((HIDDEN, L), np.float64)
    for r in res.results:
        acc += r["yT"].astype(np.float64)
    y = np.ascontiguousarray(acc.T.astype(np.float32)).reshape(1, L, HIDDEN)
    return y, res


def kernel(x, wq, wk, wv, wo):
    y, _ = _run(np.asarray(x), np.asarray(wq), np.asarray(wk),
                np.asarray(wv), np.asarray(wo))
    return y
